# revision 1
# baseline (speedup 1.0000x reference)
"""Trainium2 Bass kernel for nn_BrainGPTv2 (sparse_attention).

Sharding: (B=2, L=2048) -> 8 shards of 512 tokens; cores 0-3 own batch 0,
cores 4-7 batch 1.  Per-token work is local.  Sparse attention uses an
AllGather of importance logits per 4-core group, a rank-based top-k
(rank = #{imp > imp_t}; rank < K selects and rank is the compacted column),
matmul-based gather/scatter via an on-chip selection matrix, and a small
AllReduce of the gathered tokens.  Mamba depthwise conv runs in transposed
layout with a 3-token halo AllGather.

fp32: residual stream, LN stats, gate matmuls (rank stability).
bf16: all large matmuls (fp32 PSUM accumulation).  LN gamma/beta, res_w and
1/sqrt(hd) are folded into weights on the host.
"""
import sys

for _p in ("/opt/trn_rl_repo",):
    if _p not in sys.path:
        sys.path.append(_p)

import numpy as np
import ml_dtypes

import concourse.bass as bass
import concourse.mybir as mybir
import concourse.bacc as bacc
import concourse.tile as tile
from concourse import bass_utils
from concourse.masks import make_identity

F32 = mybir.dt.float32
BF = mybir.dt.bfloat16
I32 = mybir.dt.int32
OP = mybir.AluOpType
AF = mybir.ActivationFunctionType
AX = mybir.AxisListType

B, L, D = 2, 2048, 1024
NH, HD = 16, 64
DI, DC = 2048, 4
K = 204
LSH = 512          # tokens per core
NC_ = 8
GROUPS = [[0, 1, 2, 3], [4, 5, 6, 7]]
STEPS = 2
THRESH = 0.99
EPS = 1e-5
KC2 = [(0, 128), (128, K - 128)]   # K=204 row chunks

bf16 = ml_dtypes.bfloat16


def _r(ap, pat, **kw):
    return ap.rearrange(pat, **kw)


def build_program(b2_hp: float, vb_nonzero: bool, dbg: bool = False, nocc: bool = False,
                  qkb_zero: bool = False, inb_zero: bool = False):
    nc = bacc.Bacc("TRN2", target_bir_lowering=False, debug=False, num_devices=NC_)

    def din(name, shape, dt):
        return nc.dram_tensor(name, list(shape), dt, kind="ExternalInput")

    x_in = din("x_sh", (LSH, D), F32)
    selb_in = din("selb", (128, 4), F32)
    wqk_in = din("wqk_t", (D, 2048), BF)
    wv_in = din("wv_t", (D, D), BF)
    wo_in = din("wo_t", (D, D), BF)
    w1_in = [din(f"w1t{i}", (D, 4 * D), BF) for i in range(2)]
    w2_in = [din(f"w2t{i}", (4 * D, D), BF) for i in range(2)]
    inw_in = din("inw_t", (D, 2 * DI), BF)
    mow_in = din("moutw_t", (DI, D), BF)
    iw1_in = din("iw1t", (D, 256), F32)
    iw2_in = din("iw2c", (128, 2), F32)
    hw1_in = din("hw1t", (D, 256), F32)
    hw2_in = din("hw2c", (128, 2), F32)
    ib1_in = din("ib1c", (128, 2), F32)
    hb1_in = din("hb1c", (128, 2), F32)
    qkb_in = din("qkbc", (128, 16), F32)
    inb_in = din("inbc", (128, 32), F32)
    gel_in = [din(f"gelc{i}", (128, 32), F32) for i in range(2)]
    cw_in = din("convwc", (128, 16, 4), F32)
    dtw_in = din("dtwc", (128, 16), F32)
    dtb_in = din("dtbc", (128, 16), F32)
    dpar_in = din("dparc", (128, 16), F32)
    dtiw_in = din("dtinwc", (128, 16), BF)
    cwb_in = din("convwb", (128, 16, 4), BF)
    vb_in = din("vbrow", (1, D), F32) if vb_nonzero else None

    y_out = nc.dram_tensor("y_sh", [LSH, D], F32, kind="ExternalOutput")
    dbg_t = {}
    if dbg:
        for nm, shp, dt_ in (("d_ln1", (LSH, D), F32), ("d_impT", (1, LSH), F32),
                             ("d_impbc", (1, L), F32), ("d_ranks", (128, 4), F32),
                             ("d_tokT", (D, K), BF), ("d_qkT", (2048, K), BF),
                             ("d_oaT", (D, K), BF), ("d_xattn", (LSH, D), F32),
                             ("d_xmlp0", (LSH, D), F32), ("d_xmamba", (LSH, D), F32),
                             ("d_xiE", (16 * 128, 515), BF), ("d_p", (128, 4), F32), ("d_icols", (128, 4), F32)):
            dbg_t[nm] = nc.dram_tensor(nm, list(shp), dt_, kind="ExternalOutput")

    cc_imp_i = [nc.dram_tensor(f"cc_imp_i{s}", [1, LSH], F32, kind="Internal") for s in range(STEPS)]
    cc_imp_o = [nc.dram_tensor(f"cc_imp_o{s}", [4, LSH], F32, kind="Internal") for s in range(STEPS)]
    cc_tok_i = [nc.dram_tensor(f"cc_tok_i{s}", [D, K], BF, kind="Internal") for s in range(STEPS)]
    cc_tok_o = [nc.dram_tensor(f"cc_tok_o{s}", [D, K], BF, kind="Internal") for s in range(STEPS)]
    cc_tl_i = [nc.dram_tensor(f"cc_tl_i{s}", [DI, 3], BF, kind="Internal") for s in range(STEPS)]
    cc_tl_o = [nc.dram_tensor(f"cc_tl_o{s}", [4, DI, 3], BF, kind="Internal") for s in range(STEPS)]

    from contextlib import ExitStack
    with tile.TileContext(nc) as tc, ExitStack() as est:
        per = est.enter_context(tc.tile_pool(name="persist", bufs=1))
        wp = est.enter_context(tc.tile_pool(name="wpool", bufs=2))
        a1 = est.enter_context(tc.tile_pool(name="act", bufs=1))
        sp = est.enter_context(tc.tile_pool(name="small", bufs=2))
        s1 = est.enter_context(tc.tile_pool(name="small1", bufs=1))
        ps5 = est.enter_context(tc.tile_pool(name="ps5", bufs=3, space="PSUM"))
        ps2 = est.enter_context(tc.tile_pool(name="ps2", bufs=3, space="PSUM"))
        pst = est.enter_context(tc.tile_pool(name="pst", bufs=2, space="PSUM"))

        def _scopy(o, i):
            return nc.scalar.copy(o, i)

        def _vcopy(o, i):
            return nc.vector.tensor_copy(o, i)

        eng = [_scopy, _vcopy]

        # ---------- persistent / constants ----------
        x3 = per.tile([128, 4, D], F32, tag="x3")
        acc3 = per.tile([128, 4, D], F32, tag="acc3")
        rem = per.tile([128, 4], F32, tag="rem")
        junk = per.tile([128, 4, D], F32, tag="junk")
        idf = per.tile([128, 128], F32, tag="idf")
        idb = per.tile([128, 128], BF, tag="idb")
        iota204 = per.tile([128, K], F32, tag="iota204")
        iota_i = per.tile([128, K], I32, tag="iota_i")

        for cv in sorted({0.0, 1.0, EPS, float(-b2_hp)}):
            cvt = per.tile([128, 1], F32, tag=f"cst{cv}", name=f"cst_{cv}")
            nc.vector.memset(cvt[:], cv)
            nc.const_aps.aps[(F32, cv)] = cvt[:]

        nc.sync.dma_start(x3[:], _r(x_in[:], "(c p) d -> p c d", p=128))
        nc.vector.memset(acc3[:], 0.0)
        make_identity(nc, idf[:])
        make_identity(nc, idb[:])
        nc.gpsimd.iota(iota_i[:], pattern=[[1, K]], base=0, channel_multiplier=0)
        nc.vector.tensor_copy(iota204[:], iota_i[:])

        selb = per.tile([128, 4], F32, tag="selb")
        iw2 = per.tile([128, 2], F32, tag="iw2")
        hw2 = per.tile([128, 2], F32, tag="hw2")
        ib1 = per.tile([128, 2], F32, tag="ib1")
        hb1 = per.tile([128, 2], F32, tag="hb1")
        qkb = per.tile([128, 16], F32, tag="qkb")
        inb = per.tile([128, 32], F32, tag="inb")
        gel = [per.tile([128, 32], F32, tag=f"gel{i}", name=f"gel{i}") for i in range(2)]
        cw = per.tile([128, 16, 4], F32, tag="cw")
        dtw = per.tile([128, 16], F32, tag="dtw")
        dtb = per.tile([128, 16], F32, tag="dtb")
        dpar = per.tile([128, 16], F32, tag="dpar")
        dtiw = per.tile([128, 16], BF, tag="dtiw")
        cwb = per.tile([128, 16, 4], BF, tag="cwb")
        for t, d in ((selb, selb_in), (iw2, iw2_in), (hw2, hw2_in), (ib1, ib1_in),
                     (hb1, hb1_in), (qkb, qkb_in), (inb, inb_in),
                     (gel[0], gel_in[0]), (gel[1], gel_in[1]), (cw, cw_in),
                     (dtw, dtw_in), (dtb, dtb_in), (dpar, dpar_in), (dtiw, dtiw_in),
                     (cwb, cwb_in)):
            nc.sync.dma_start(t[:], d[:])
        if vb_nonzero:
            vbr = per.tile([1, D], F32, tag="vbr")
            nc.sync.dma_start(vbr[:], vb_in[:])

        # ---------- helpers ----------
        def ln_norm():
            """raw layernorm of x3 -> junk (f32)."""
            st = sp.tile([128, 4, 5], F32, tag="lnst", name="lnst")
            s_, sq = st[:, :, 0], st[:, :, 1]
            mean, rstd, nmr = st[:, :, 2], st[:, :, 3], st[:, :, 4]
            nc.vector.reduce_sum(s_, x3[:], axis=AX.X)
            nc.scalar.square(junk[:], x3[:])
            nc.vector.reduce_sum(sq, junk[:], axis=AX.X)
            nc.vector.tensor_scalar_mul(mean, s_, 1.0 / D)
            nc.vector.tensor_mul(s_, mean, mean)
            nc.vector.scalar_tensor_tensor(sq, sq, 1.0 / D, s_, op0=OP.mult, op1=OP.subtract)
            nc.scalar.activation(rstd, sq, AF.Sqrt, bias=EPS, scale=1.0)
            nc.vector.reciprocal(rstd, rstd)
            nc.vector.scalar_tensor_tensor(nmr, mean, -1.0, rstd, op0=OP.mult, op1=OP.mult)
            for c in range(4):
                nc.vector.tensor_scalar(junk[:, c, :], x3[:, c, :],
                                        rstd[:, c:c + 1], nmr[:, c:c + 1],
                                        op0=OP.mult, op1=OP.add)

        def transpose_LD(src3, dst3, ident, ei=0):
            """src3 (128,4,D) -> dst3 (128,8,512), cast on psum->sbuf copy"""
            for dc in range(8):
                pt = pst.tile([128, 512], src3.tensor.dtype, tag="pt", name=f"pt{dc}")
                for c in range(4):
                    nc.tensor.transpose(pt[:, c * 128:(c + 1) * 128],
                                        src3[:, c, dc * 128:(dc + 1) * 128], ident[:])
                eng[(dc + ei) % 2](dst3[:, dc, :], pt[:])

        def gate_from_x(src3, w1sb, b1col, w2col, out_cols, sig_bias=None):
            """fp32 gate on src3 (128,4,D): h=relu(T(src)@w1+b1); out = h@w2."""
            hps = [ps5.tile([128, 512], F32, tag="mm5", name=f"hps{i}") for i in range(2)]
            for dc in range(8):
                pt = pst.tile([128, 512], F32, tag="pt", name=f"gpt{dc}")
                for c in range(4):
                    nc.tensor.transpose(pt[:, c * 128:(c + 1) * 128],
                                        src3[:, c, dc * 128:(dc + 1) * 128], idf[:])
                tTc = sp.tile([128, 512], F32, tag="tTc", name=f"tTc{dc}")
                eng[dc % 2](tTc[:], pt[:])
                for oc in range(2):
                    nc.tensor.matmul(hps[oc][:], w1sb[:, dc, oc * 128:(oc + 1) * 128],
                                     tTc[:], start=dc == 0, stop=dc == 7)
            h = s1.tile([128, 2, 512], F32, tag="gateh", name="gateh")
            for oc in range(2):
                nc.scalar.activation(h[:, oc, :], hps[oc][:], AF.Relu, bias=b1col[:, oc:oc + 1])
            if out_cols.shape[-1] == 4:
                for lc in range(4):
                    ps = ps2.tile([128, K], F32, tag="mm2", name=f"gp{lc}")
                    for oc in range(2):
                        nc.tensor.matmul(ps[:, 0:1], h[:, oc, lc * 128:(lc + 1) * 128],
                                         w2col[:, oc:oc + 1], start=oc == 0, stop=oc == 1)
                    nc.scalar.activation(out_cols[:, lc:lc + 1], ps[:, 0:1], AF.Exp,
                                         bias=float(-sig_bias), scale=-1.0)
                nc.vector.tensor_scalar_add(out_cols[:], out_cols[:], 1.0)
                nc.vector.reciprocal(out_cols[:], out_cols[:])
            else:
                ps = ps5.tile([128, 512], F32, tag="mm5", name="gpi")
                for oc in range(2):
                    nc.tensor.matmul(ps[:1, :], w2col[:, oc:oc + 1], h[:, oc, :],
                                     start=oc == 0, stop=oc == 1)
                nc.vector.tensor_copy(out_cols[:], ps[:1, :])

        # ================= per-step program =================
        for s in range(STEPS):
            zT = a1.tile([128, 16, 512], BF, tag="m16b", name=f"zT_{s}")

            # ---------- block 0 : selective attention ----------
            ln_norm()                                      # junk := ln1(x)

            if dbg and s == 0:
                nc.sync.dma_start(_r(dbg_t["d_ln1"][:], "(c p) d -> p c d", p=128), junk[:])
            iw1 = wp.tile([128, 8, 256], F32, tag="wbig", name=f"iw1_{s}")
            nc.sync.dma_start(iw1[:], _r(iw1_in[:], "(k p) o -> p k o", p=128))
            impT = sp.tile([1, 512], F32, tag="impT", name=f"impT_{s}")
            gate_from_x(junk, iw1, ib1, iw2, impT)

            nc.sync.dma_start(cc_imp_i[s][:], impT[:])
            if nocc:
                for g_ in range(4):
                    nc.sync.dma_start(cc_imp_o[s][g_:g_ + 1, :], cc_imp_i[s][:])
            else:
                nc.gpsimd.collective_compute("AllGather", OP.bypass, replica_groups=GROUPS,
                                             ins=[cc_imp_i[s][:]], outs=[cc_imp_o[s][:]])
            imp_bc = a1.tile([128, L], F32, tag="impbc", name=f"impbc_{s}")
            nc.sync.dma_start(imp_bc[:],
                              _r(cc_imp_o[s][:], "g l -> (g l)").unsqueeze(0).broadcast_to([128, L]))

            if dbg and s == 0:
                nc.sync.dma_start(dbg_t["d_impT"][:], impT[:])
                nc.sync.dma_start(dbg_t["d_impbc"][:], imp_bc[:1, :])
            pic = pst.tile([128, 512], F32, tag="pt", name=f"pic_{s}")
            for c in range(4):
                nc.tensor.transpose(pic[:, c:c + 1], impT[:1, c * 128:(c + 1) * 128],
                                    idf[:1, :1])
            icols = sp.tile([128, 4], F32, tag="icols", name=f"icols_{s}")
            nc.vector.tensor_copy(icols[:], pic[:, 0:4])
            if dbg and s == 0:
                nc.sync.dma_start(dbg_t["d_icols"][:], icols[:])
            ranks = sp.tile([128, 4], F32, tag="ranks", name=f"ranks_{s}")
            zflat = _r(zT[:], "p a b -> p (a b)")
            for c in range(4):
                nc.vector.tensor_scalar(zflat[:, c * L:(c + 1) * L], imp_bc[:],
                                        icols[:, c:c + 1], None, op0=OP.is_gt)
                nc.vector.reduce_sum(ranks[:, c:c + 1],
                                     zflat[:, c * L:(c + 1) * L], axis=AX.X)

            if dbg and s == 0:
                nc.sync.dma_start(dbg_t["d_ranks"][:], ranks[:])
            # selection matrices (fp32 ST pairs with fp32 normalized x)
            ST = a1.tile([128, 4, K], F32, tag="ST", name=f"ST_{s}")
            for c in range(4):
                nc.vector.tensor_scalar(ST[:, c, :], iota204[:], ranks[:, c:c + 1], None,
                                        op0=OP.is_equal)
            S_ = a1.tile([128, 2, 512], BF, tag="S_", name=f"S__{s}")
            for c in range(4):
                pt = pst.tile([128, 2, 128], F32, tag="pt", name=f"ptS{c}")
                nc.tensor.transpose(pt[:, 0, :], ST[:, c, 0:128], idf[:])
                nc.tensor.transpose(pt[:76, 1, :], ST[:, c, 128:K], idf[:])
                nc.vector.tensor_copy(S_[:, :, c * 128:(c + 1) * 128], pt[:])

            # gather own tokens -> bf16 partial tokT, AllReduce
            tokp = a1.tile([128, 8, K], BF, tag="tokp", name=f"tokp_{s}")
            for dc in range(8):
                ps = ps2.tile([128, K], F32, tag="mm2", name=f"tg{dc}")
                for c in range(4):
                    nc.tensor.matmul(ps[:], junk[:, c, dc * 128:(dc + 1) * 128],
                                     ST[:, c, :], start=c == 0, stop=c == 3)
                eng[dc % 2](tokp[:, dc, :], ps[:])
            nc.sync.dma_start(_r(cc_tok_i[s][:], "(dc p) k -> p dc k", p=128), tokp[:])
            if nocc:
                nc.sync.dma_start(cc_tok_o[s][:], cc_tok_i[s][:])
            else:
                nc.gpsimd.collective_compute("AllReduce", OP.add, replica_groups=GROUPS,
                                             ins=[cc_tok_i[s][:]], outs=[cc_tok_o[s][:]])
            tokT = a1.tile([128, 8, K], BF, tag="tokT", name=f"tokT_{s}")
            nc.sync.dma_start(tokT[:], _r(cc_tok_o[s][:], "(dc p) k -> p dc k", p=128))

            if dbg and s == 0:
                nc.sync.dma_start(_r(dbg_t["d_tokT"][:], "(dc p) k -> p dc k", p=128), tokT[:])
            # qkT (+bias)
            qkT = a1.tile([128, 16, K], BF, tag="qkT", name=f"qkT_{s}")
            for half in range(2):
                wqk = wp.tile([128, 8, 1024], BF, tag="wbig", name=f"wqk{half}_{s}")
                nc.sync.dma_start(wqk[:], _r(wqk_in[:, half * 1024:(half + 1) * 1024],
                                             "(k p) o -> p k o", p=128))
                for oc in range(8):
                    ps = ps2.tile([128, K], F32, tag="mm2", name=f"qk{oc}")
                    for kc in range(8):
                        nc.tensor.matmul(ps[:], wqk[:, kc, oc * 128:(oc + 1) * 128],
                                         tokT[:, kc, :], start=kc == 0, stop=kc == 7)
                    if qkb_zero:
                        eng[oc % 2](qkT[:, half * 8 + oc, :], ps[:])
                    else:
                        nc.scalar.activation(qkT[:, half * 8 + oc, :], ps[:], AF.Identity,
                                             bias=qkb[:, half * 8 + oc:half * 8 + oc + 1])
            if dbg and s == 0:
                nc.sync.dma_start(_r(dbg_t["d_qkT"][:], "(dc p) k -> p dc k", p=128), qkT[:])
            # v
            wv = wp.tile([128, 8, D], BF, tag="wbig", name=f"wv_{s}")
            nc.sync.dma_start(wv[:], _r(wv_in[:], "(k p) o -> p k o", p=128))
            v_ = [a1.tile([rn, D], BF, tag=f"v{i}", name=f"v{i}_{s}") for i, (rs, rn) in enumerate(KC2)]
            for i, (rs, rn) in enumerate(KC2):
                for hf in range(2):
                    ps = ps5.tile([128, 512], F32, tag="mm5", name=f"vp{i}{hf}")
                    for dc in range(8):
                        nc.tensor.matmul(ps[:rn, :], tokT[:, dc, rs:rs + rn],
                                         wv[:, dc, hf * 512:(hf + 1) * 512],
                                         start=dc == 0, stop=dc == 7)
                    if vb_nonzero:
                        nc.vector.tensor_add(ps[:rn, :], ps[:rn, :],
                                             vbr[:1, hf * 512:(hf + 1) * 512].broadcast_to([rn, 512]))
                    eng[(i + hf) % 2](v_[i][:, hf * 512:(hf + 1) * 512], ps[:rn, :])

            # attention heads
            oaT = a1.tile([128, 8, K], BF, tag="oaT", name=f"oaT_{s}")
            for h in range(NH):
                hp_, hs = h // 2, 64 * (h % 2)
                attb = [sp.tile([rn, K], BF, tag=f"attb{i}", name=f"attb{i}_{s}_{h}")
                        for i, (rs, rn) in enumerate(KC2)]
                for i, (qs, qn) in enumerate(KC2):
                    ps = ps2.tile([128, K], F32, tag="mm2", name=f"sc{i}")
                    nc.tensor.matmul(ps[:qn, :], qkT[hs:hs + 64, hp_, qs:qs + qn],
                                     qkT[hs:hs + 64, 8 + hp_, :], start=True, stop=True)
                    mx = sp.tile([128, 2], F32, tag="mx", name=f"mx{i}_{s}_{h}")
                    nc.vector.tensor_reduce(mx[:qn, 0:1], ps[:qn, :], axis=AX.X,
                                            op=OP.max, negate=True)
                    nc.scalar.activation(attb[i][:], ps[:qn, :], AF.Exp,
                                         bias=mx[:qn, 0:1], scale=1.0,
                                         accum_out=mx[:qn, 1:2])
                    nc.vector.reciprocal(mx[:qn, 0:1], mx[:qn, 1:2])
                    nc.vector.tensor_scalar_mul(attb[i][:], attb[i][:], mx[:qn, 0:1])
                attT = sp.tile([128, 2, K], BF, tag="attT", name=f"attT_{s}_{h}")
                pt = pst.tile([128, 2, K], BF, tag="pt", name=f"ptA_{s}_{h}")
                nc.tensor.transpose(pt[:, 0, 0:128], attb[0][:, 0:128], idb[:])
                nc.tensor.transpose(pt[:76, 1, 0:128], attb[0][:, 128:K], idb[:])
                nc.tensor.transpose(pt[:, 0, 128:K], attb[1][:, 0:128], idb[:76, :76])
                nc.tensor.transpose(pt[:76, 1, 128:K], attb[1][:, 128:K], idb[:76, :76])
                nc.vector.tensor_copy(attT[:], pt[:])
                po = ps2.tile([128, K], F32, tag="mm2", name=f"po_{h}")
                nc.tensor.matmul(po[:64, :], v_[0][:, h * 64:(h + 1) * 64],
                                 attT[:, 0, :], start=True, stop=False)
                nc.tensor.matmul(po[:64, :], v_[1][:76, h * 64:(h + 1) * 64],
                                 attT[:76, 1, :], start=False, stop=True)
                eng[h % 2](oaT[hs:hs + 64, hp_, :], po[:64, :])

            if dbg and s == 0:
                nc.sync.dma_start(_r(dbg_t["d_oaT"][:], "(dc p) k -> p dc k", p=128), oaT[:])
            # output projection + scatter + residual
            wo = wp.tile([128, 8, D], BF, tag="wbig", name=f"wo_{s}")
            nc.sync.dma_start(wo[:], _r(wo_in[:], "(k p) o -> p k o", p=128))
            o_ = [a1.tile([rn, D], BF, tag=f"o{i}", name=f"o{i}_{s}") for i, (rs, rn) in enumerate(KC2)]
            for i, (rs, rn) in enumerate(KC2):
                for hf in range(2):
                    ps = ps5.tile([128, 512], F32, tag="mm5", name=f"op{i}{hf}")
                    for dc in range(8):
                        nc.tensor.matmul(ps[:rn, :], oaT[:, dc, rs:rs + rn],
                                         wo[:, dc, hf * 512:(hf + 1) * 512],
                                         start=dc == 0, stop=dc == 7)
                    eng[(i + hf) % 2](o_[i][:, hf * 512:(hf + 1) * 512], ps[:rn, :])
            for lc in range(4):
                for hf in range(2):
                    ps = ps5.tile([128, 512], F32, tag="mm5", name=f"scat{lc}{hf}")
                    nc.tensor.matmul(ps[:], S_[:, 0, lc * 128:(lc + 1) * 128],
                                     o_[0][:, hf * 512:(hf + 1) * 512], start=True, stop=False)
                    nc.tensor.matmul(ps[:], S_[:76, 1, lc * 128:(lc + 1) * 128],
                                     o_[1][:76, hf * 512:(hf + 1) * 512], start=False, stop=True)
                    # mixer returns ln1(x) + res*scat; block adds it to x
                    nc.vector.tensor_add(ps[:], ps[:],
                                         junk[:, lc, hf * 512:(hf + 1) * 512])
                    nc.vector.tensor_add(x3[:, lc, hf * 512:(hf + 1) * 512],
                                         x3[:, lc, hf * 512:(hf + 1) * 512], ps[:])

            if dbg and s == 0:
                nc.sync.dma_start(_r(dbg_t["d_xattn"][:], "(c p) d -> p c d", p=128), x3[:])
            # ---------- MLP (after both mixers) ----------
            def mlp(bi):
                ln_norm()                                  # junk := ln2(x)
                uT = a1.tile([128, 8, 512], BF, tag="uT", name=f"uT_{s}_{bi}")
                transpose_LD(junk, uT, idf)
                for half in range(2):                      # 2048 hidden ch at a time
                    gT = a1.tile([128, 16, 512], BF, tag="gT", name=f"gT_{s}_{bi}_{half}")
                    for q in range(2):
                        w1u = wp.tile([128, 8, 1024], BF, tag="wbig",
                                      name=f"w1u_{s}_{bi}_{half}_{q}")
                        nc.sync.dma_start(
                            w1u[:], _r(w1_in[bi][:, (half * 2 + q) * 1024:(half * 2 + q + 1) * 1024],
                                       "(k p) o -> p k o", p=128))
                        for oc in range(8):
                            ps = ps5.tile([128, 512], F32, tag="mm5", name=f"up{oc}")
                            for kc in range(8):
                                nc.tensor.matmul(ps[:], w1u[:, kc, oc * 128:(oc + 1) * 128],
                                                 uT[:, kc, :], start=kc == 0, stop=kc == 7)
                            och = half * 16 + q * 8 + oc
                            nc.scalar.activation(gT[:, q * 8 + oc, :], ps[:], AF.Gelu,
                                                 bias=gel[bi][:, och:och + 1])
                    for hf in range(2):
                        w2u = wp.tile([128, 16, 512], BF, tag="wbig",
                                      name=f"w2u_{s}_{bi}_{half}_{hf}")
                        nc.sync.dma_start(
                            w2u[:], _r(w2_in[bi][half * 2048:(half + 1) * 2048,
                                                 hf * 512:(hf + 1) * 512],
                                       "(k p) o -> p k o", p=128))
                        for lc in range(4):
                            ps = ps5.tile([128, 512], F32, tag="mm5", name=f"dn{lc}")
                            for kc in range(16):
                                nc.tensor.matmul(ps[:], gT[:, kc, lc * 128:(lc + 1) * 128],
                                                 w2u[:, kc, :], start=kc == 0, stop=kc == 15)
                            nc.vector.tensor_add(x3[:, lc, hf * 512:(hf + 1) * 512],
                                                 x3[:, lc, hf * 512:(hf + 1) * 512], ps[:])

            mlp(0)

            if dbg and s == 0:
                nc.sync.dma_start(_r(dbg_t["d_xmlp0"][:], "(c p) d -> p c d", p=128), x3[:])
            # ---------- block 1 : mamba ----------
            ln_norm()                                      # junk := ln1b(x)
            t2T = a1.tile([128, 8, 512], BF, tag="uT", name=f"t2T_{s}")
            transpose_LD(junk, t2T, idf)
            xiE = a1.tile([128, 16, 515], BF, tag="m16a", name=f"xiE_{s}")
            tailsb = sp.tile([128, 16, 3], BF, tag="tailsb", name=f"tailsb_{s}")
            for u in range(4):
                inwu = wp.tile([128, 8, 1024], BF, tag="wbig", name=f"inw{u}_{s}")
                nc.sync.dma_start(inwu[:], _r(inw_in[:, u * 1024:(u + 1) * 1024],
                                              "(k p) o -> p k o", p=128))
                if u < 2:
                    # xi for the last 3 own tokens, sent ahead so the halo
                    # AllGather overlaps the rest of the in-projection
                    for oc8 in range(8):
                        oc = u * 8 + oc8
                        pt3 = ps2.tile([128, K], F32, tag="mm2", name=f"tl{oc8}")
                        for kc in range(8):
                            nc.tensor.matmul(pt3[:, 0:3],
                                             inwu[:, kc, oc8 * 128:(oc8 + 1) * 128],
                                             t2T[:, kc, 509:512],
                                             start=kc == 0, stop=kc == 7)
                        if inb_zero:
                            nc.vector.tensor_copy(tailsb[:, oc, :], pt3[:, 0:3])
                        else:
                            nc.scalar.activation(tailsb[:, oc, :], pt3[:, 0:3], AF.Identity,
                                                 bias=inb[:, oc:oc + 1])
                    if u == 1:
                        nc.sync.dma_start(_r(cc_tl_i[s][:], "(k p) j -> p k j", p=128),
                                          tailsb[:])
                        if nocc:
                            for g_ in range(4):
                                nc.sync.dma_start(cc_tl_o[s][g_], cc_tl_i[s][:])
                        else:
                            nc.gpsimd.collective_compute(
                                "AllGather", OP.bypass, replica_groups=GROUPS,
                                ins=[cc_tl_i[s][:]], outs=[cc_tl_o[s][:]])
                for oc8 in range(8):
                    oc = u * 8 + oc8
                    ps = ps5.tile([128, 512], F32, tag="mm5", name=f"ip{oc8}")
                    for kc in range(8):
                        nc.tensor.matmul(ps[:], inwu[:, kc, oc8 * 128:(oc8 + 1) * 128],
                                         t2T[:, kc, :], start=kc == 0, stop=kc == 7)
                    if oc < 16:
                        if inb_zero:
                            eng[oc % 2](xiE[:, oc, 3:515], ps[:])
                        else:
                            nc.scalar.activation(xiE[:, oc, 3:515], ps[:], AF.Identity,
                                                 bias=inb[:, oc:oc + 1])
                    else:
                        nc.scalar.activation(zT[:, oc - 16, :], ps[:], AF.Silu,
                                             bias=inb[:, oc:oc + 1])

            tails = sp.tile([128, 16, 4, 3], BF, tag="tails", name=f"tails_{s}")
            for g_ in range(4):
                nc.sync.dma_start(tails[:, :, g_, :],
                                  _r(cc_tl_o[s][:], "g (k p) j -> g p k j", p=128)[g_])
            htmp = sp.tile([128, 16, 3, 4], F32, tag="htmp", name=f"htmp_{s}")
            hsum = sp.tile([128, 16, 3], F32, tag="hsum", name=f"hsum_{s}")
            nc.vector.tensor_mul(htmp[:], tails[:].transpose([0, 1, 3, 2]),
                                 selb[:].unsqueeze(1).unsqueeze(1).broadcast_to([128, 16, 3, 4]))
            nc.vector.reduce_sum(hsum[:], htmp[:], axis=AX.X)
            nc.gpsimd.tensor_copy(xiE[:, :, 0:3], hsum[:])

            # depthwise causal conv as 4 diagonal-matmul taps + silu
            for kc in range(16):
                pc = ps5.tile([128, 512], F32, tag="mm5", name=f"cv_{kc}")
                for j in range(4):
                    dg = sp.tile([128, 128], BF, tag="diag", name=f"dg_{s}_{kc}_{j}")
                    nc.gpsimd.affine_select(
                        dg[:], cwb[:, kc, j:j + 1].broadcast_to([128, 128]),
                        pattern=[[-1, 128]], compare_op=OP.is_equal, fill=0.0,
                        base=0, channel_multiplier=1)
                    nc.tensor.matmul(pc[:, 3:512], dg[:], xiE[:, kc, 3 + j:512 + j],
                                     start=j == 0, stop=False)
                    nc.tensor.matmul(pc[:, 0:3], dg[:], xiE[:, kc, j:3 + j],
                                     start=j == 0, stop=j == 3)
                nc.scalar.activation(xiE[:, kc, 3:515], pc[:], AF.Silu)

            if dbg and s == 0:
                nc.sync.dma_start(_r(dbg_t["d_xiE"][:], "(c p) j -> p c j", p=128), xiE[:])
            # dt path + gating (result written into zT)
            psd = ps5.tile([128, 512], F32, tag="mm5", name=f"dtin_{s}")
            for kc in range(16):
                nc.tensor.matmul(psd[:1, :], dtiw[:, kc:kc + 1], xiE[:, kc, 3:515],
                                 start=kc == 0, stop=kc == 15)
            dt_bc = s1.tile([128, 512], F32, tag="dtbc", name=f"dtbc_{s}")
            nc.vector.tensor_copy(dt_bc[:1, :], psd[:1, :])
            nc.gpsimd.partition_broadcast(dt_bc[:], dt_bc[:1, :])
            for kc in range(16):
                # sigmoid(softplus(z)) = (1+u)/(2+u) with u = e^z, so
                # gate + D = (1+D) - 1/(2+u);  dparc ships 1+D.
                dsp = sp.tile([128, 512], F32, tag="dsp", name=f"dsp_{s}_{kc}")
                nc.scalar.activation(dsp[:], dt_bc[:], AF.Exp,
                                     bias=dtb[:, kc:kc + 1], scale=dtw[:, kc:kc + 1])
                nc.vector.tensor_scalar_add(dsp[:], dsp[:], 2.0)
                nc.vector.reciprocal(dsp[:], dsp[:])
                nc.vector.tensor_scalar(dsp[:], dsp[:], -1.0, dpar[:, kc:kc + 1],
                                        op0=OP.mult, op1=OP.add)
                nc.vector.tensor_mul(dsp[:], dsp[:], xiE[:, kc, 3:515])
                nc.gpsimd.tensor_mul(zT[:, kc, :], dsp[:], zT[:, kc, :])

            for hf in range(2):
                mowu = wp.tile([128, 16, 512], BF, tag="wbig", name=f"mow{hf}_{s}")
                nc.sync.dma_start(mowu[:], _r(mow_in[:, hf * 512:(hf + 1) * 512],
                                              "(k p) o -> p k o", p=128))
                for lc in range(4):
                    ps = ps5.tile([128, 512], F32, tag="mm5", name=f"mo{lc}")
                    for kc in range(16):
                        nc.tensor.matmul(ps[:], zT[:, kc, lc * 128:(lc + 1) * 128],
                                         mowu[:, kc, :], start=kc == 0, stop=kc == 15)
                    nc.vector.tensor_add(x3[:, lc, hf * 512:(hf + 1) * 512],
                                         x3[:, lc, hf * 512:(hf + 1) * 512], ps[:])

            if dbg and s == 0:
                nc.sync.dma_start(_r(dbg_t["d_xmamba"][:], "(c p) d -> p c d", p=128), x3[:])
            mlp(1)

            # ---------- halting gate (last step's gate is algebraically dead:
            # nh_last + rem_last == rem_{last-1}) ----------
            if s < STEPS - 1:
                hw1 = wp.tile([128, 8, 256], F32, tag="wbig", name=f"hw1_{s}")
                nc.sync.dma_start(hw1[:], _r(hw1_in[:], "(k p) o -> p k o", p=128))
                p_ = sp.tile([128, 4], F32, tag="pcol", name=f"p_{s}")
                gate_from_x(x3, hw1, hb1, hw2, p_, sig_bias=float(b2_hp))
                nh = sp.tile([128, 4], F32, tag="nh", name=f"nh_{s}")
                # halt starts at 0 < THRESH, rem starts at 1:
                # nh = min(1, p); rem = 1 - nh; acc += nh * x
                if dbg and s == 0:
                    nc.sync.dma_start(dbg_t["d_p"][:], p_[:])
                nc.vector.tensor_scalar_min(nh[:], p_[:], 1.0)
                nc.vector.tensor_scalar(rem[:], nh[:], -1.0, 1.0, op0=OP.mult, op1=OP.add)
                for c in range(4):
                    nc.vector.scalar_tensor_tensor(acc3[:, c, :], x3[:, c, :],
                                                   nh[:, c:c + 1], acc3[:, c, :],
                                                   op0=OP.mult, op1=OP.add)

        # ---------- final output: y = acc + rem * x ----------
        for c in range(4):
            nc.vector.scalar_tensor_tensor(junk[:, c, :], x3[:, c, :], rem[:, c:c + 1],
                                           acc3[:, c, :], op0=OP.mult, op1=OP.add)
        nc.sync.dma_start(_r(y_out[:], "(c p) d -> p c d", p=128), junk[:])

    nc.compile()
    return nc


_CACHE = {}


def _get_program(b2_hp, vb_nonzero, qkb_zero=False, inb_zero=False):
    key = (round(float(b2_hp), 9), bool(vb_nonzero), qkb_zero, inb_zero)
    if key not in _CACHE:
        _CACHE[key] = build_program(float(b2_hp), bool(vb_nonzero),
                                    qkb_zero=qkb_zero, inb_zero=inb_zero)
    return _CACHE[key]


def _cols(a, n=128):
    """(n*k,) -> (n, k) column-chunk layout"""
    a = np.asarray(a, np.float32).reshape(-1, n)
    return np.ascontiguousarray(a.T)


def prepare_inputs(inp):
    g = lambda k: np.asarray(inp[k], np.float32)
    x = g("x")

    def fold(w, gam, bet):
        return w * gam[None, :], w @ bet

    iw1, ib1 = fold(g("b0_imp_w1"), g("b0_ln1_g"), g("b0_ln1_b"))
    ib1 = ib1 + g("b0_imp_b1")
    qkv, qkvb = fold(g("b0_qkv_w"), g("b0_ln1_g"), g("b0_ln1_b"))
    qkv = qkv.copy()
    qkvb = qkvb.copy()
    qkv[:D] /= np.sqrt(HD)
    qkvb[:D] /= np.sqrt(HD)
    w10, gel0 = fold(g("b0_mlp_w1"), g("b0_ln2_g"), g("b0_ln2_b"))
    gel0 = gel0 + 0.0
    inw, inb = fold(g("b1_in_w"), g("b1_ln1_g"), g("b1_ln1_b"))
    w11, gel1 = fold(g("b1_mlp_w1"), g("b1_ln2_g"), g("b1_ln2_b"))
    wo = float(g("b0_res_w")[0]) * g("b0_out_w")

    vb = qkvb[2 * D:]
    vb_nonzero = bool(np.any(vb != 0.0))

    wdict = {
        "wqk_t": np.ascontiguousarray(qkv[:2 * D].T).astype(bf16),
        "wv_t": np.ascontiguousarray(qkv[2 * D:].T).astype(bf16),
        "wo_t": np.ascontiguousarray(wo.T).astype(bf16),
        "w1t0": np.ascontiguousarray(w10.T).astype(bf16),
        "w2t0": np.ascontiguousarray(g("b0_mlp_w2").T).astype(bf16),
        "w1t1": np.ascontiguousarray(w11.T).astype(bf16),
        "w2t1": np.ascontiguousarray(g("b1_mlp_w2").T).astype(bf16),
        "inw_t": np.ascontiguousarray(inw.T).astype(bf16),
        "moutw_t": np.ascontiguousarray(g("b1_out_w").T).astype(bf16),
        "iw1t": np.ascontiguousarray(iw1.T),
        "iw2c": _cols(g("b0_imp_w2")[0]),
        "hw1t": np.ascontiguousarray(g("hp_w1").T),
        "hw2c": _cols(g("hp_w2")[0]),
        "ib1c": _cols(ib1),
        "hb1c": _cols(g("hp_b1")),
        "qkbc": _cols(qkvb[:2 * D]),
        "inbc": _cols(inb),
        "gelc0": _cols(gel0),
        "gelc1": _cols(gel1),
        "convwc": np.ascontiguousarray(
            g("b1_conv_w").reshape(DI, DC).reshape(16, 128, DC).transpose(1, 0, 2)),
        "dtwc": _cols(g("b1_dt_w")[:, 0]),
        "dtbc": _cols(g("b1_dt_b")),
        "dparc": _cols(1.0 + g("b1_D")),
        "dtinwc": _cols(g("b1_xproj_w")[2 * 16]).astype(bf16),
        "convwb": np.ascontiguousarray(
            g("b1_conv_w").reshape(DI, DC).reshape(16, 128, DC).transpose(1, 0, 2)).astype(bf16),
    }
    if vb_nonzero:
        wdict["vbrow"] = np.ascontiguousarray(vb[None, :])

    in_maps = []
    for core in range(NC_):
        b, gp = core // 4, core % 4
        sel = np.zeros(4, np.float32)
        if gp > 0:
            sel[gp - 1] = 1.0
        m = dict(wdict)
        m["x_sh"] = np.ascontiguousarray(x[b, gp * LSH:(gp + 1) * LSH])
        m["selb"] = np.ascontiguousarray(np.broadcast_to(sel, (128, 4)).copy())
        in_maps.append(m)
    flags = (bool(not np.any(qkvb[:2 * D])), bool(not np.any(inb)))
    return in_maps, float(g("hp_b2")[0]), vb_nonzero, flags


def kernel(**inputs):
    in_maps, b2_hp, vb_nonzero, flags = prepare_inputs(inputs)
    nc = _get_program(b2_hp, vb_nonzero, *flags)
    res = bass_utils.run_bass_kernel_spmd(nc, in_maps, core_ids=list(range(NC_)))
    y = np.zeros((B, L, D), np.float32)
    for core in range(NC_):
        b, gp = core // 4, core % 4
        y[b, gp * LSH:(gp + 1) * LSH] = res.results[core]["y_sh"]
    return y



# revision 20
# speedup vs baseline: 1.2554x; 1.2554x over previous
"""Trainium2 Bass kernel for nn_BrainGPTv2 (sparse_attention).

Sharding: (B=2, L=2048) -> 8 shards of 512 tokens; cores 0-3 own batch 0,
cores 4-7 batch 1.  Per-token work is local.  Sparse attention uses an
AllGather of importance logits per 4-core group, a rank-based top-k
(rank = #{imp > imp_t}; rank < K selects and rank is the compacted column),
matmul-based gather/scatter via an on-chip selection matrix, and a small
AllReduce of the gathered tokens.  Mamba depthwise conv runs in transposed
layout with a 3-token halo AllGather.

Optimizations over the first working version:
- host-side weight layouts match SBUF destination exactly (contiguous DMA)
- collective-adjacent DMAs moved off the sync HWDGE FIFO (scalar/gpsimd
  queues) so weight streaming is never head-of-line blocked by a collective
- LN rstd via quake-style rsqrt on DVE (no scalar-engine act-table thrash);
  LN stats split across vector (sumsq) and gpsimd (sum)
- importance gate returns raw logits (sigmoid is rank-monotone -> dropped);
  gate w1 matmuls in bf16
- rank computation fused to one DVE pass per chunk via accum_out
- attention: scores computed pre-transposed (K-major), exp without max-sub,
  softmax normalization applied after the V contraction (removes 4 PE
  transposes + max-reduce per head)
- mamba dt gate: sigmoid(softplus(z))+D == (0.75+D) - 0.25*tanh((ln2-z)/2)
  -> one scalar Tanh + one DVE op per channel group (removes 32 full-width
  DVE reciprocals)
- ln1(x) residual re-add folded into x3 via (1+rstd) scale (lets the
  normalized copy be bf16-only; gather matmuls run in bf16)
- conv diagonal taps generated once per step into the (dead) gT slot
- mamba halo tails computed before the main in-projection so the halo
  AllGather overlaps all of it

fp32: residual stream, LN stats, gate w2/h, rank values.
bf16: all large matmuls (fp32 PSUM accumulation).
"""
import sys

for _p in ("/opt/trn_rl_repo",):
    if _p not in sys.path:
        sys.path.append(_p)

import numpy as np
import ml_dtypes

import concourse.bass as bass
import concourse.mybir as mybir
import concourse.bacc as bacc
import concourse.tile as tile
from concourse import bass_utils
from concourse.masks import make_identity

F32 = mybir.dt.float32
U32 = mybir.dt.uint32
BF = mybir.dt.bfloat16
I32 = mybir.dt.int32
OP = mybir.AluOpType
AF = mybir.ActivationFunctionType
AX = mybir.AxisListType

B, L, D = 2, 2048, 1024
NH, HD = 16, 64
DI, DC = 2048, 4
K = 204
LSH = 512          # tokens per core
NC_ = 8
GROUPS = [[0, 1, 2, 3], [4, 5, 6, 7]]
STEPS = 2
THRESH = 0.99
EPS = 1e-5
LN2C = 0.6931471805599453
MAGIC = 0x5F3759DF
KC2 = [(0, 128), (128, K - 128)]   # K=204 row chunks

bf16 = ml_dtypes.bfloat16


def _r(ap, pat, **kw):
    return ap.rearrange(pat, **kw)


import os

QUEUE_SPLIT = os.environ.get("KQ", "1") == "1"   # collective DMAs off the sync FIFO
ATT_NEW = os.environ.get("KATT", "1") == "1"     # restructured attention softmax
KLN = os.environ.get("KLN", "1")                 # 1=bn_stats+quake 2=bn only 3=quake only 0=neither
LN_TTR = KLN in ("1", "2")
LN_QUAKE = KLN in ("1", "3")
# NOTE: in-place DVE tensor_scalar on x3 faults the device (NRT status 101);
# the ln1-residual fold is permanently disabled — scatter adds junkb instead.
RES_NEW = os.environ.get("KRES", "0") == "1"


def build_program(b2_hp: float, vb_nonzero: bool, dbg: bool = False, nocc: bool = False,
                  qkb_zero: bool = False, inb_zero: bool = False):
    nc = bacc.Bacc("TRN2", target_bir_lowering=False, debug=False, num_devices=NC_)
    sdma = nc.scalar if QUEUE_SPLIT else nc.sync
    gdma = nc.gpsimd if QUEUE_SPLIT else nc.sync

    def din(name, shape, dt):
        return nc.dram_tensor(name, list(shape), dt, kind="ExternalInput")

    x_in = din("x_sh", (128, 4, D), F32)
    selb_in = din("selb", (128, 4), F32)
    wqk_in = [din(f"wqk{h}", (128, 8, 1024), BF) for h in range(2)]
    wv_in = din("wv_t", (128, 8, 1024), BF)
    wo_in = din("wo_t", (128, 8, 1024), BF)
    w1_in = [[din(f"w1q{i}_{j}", (128, 8, 1024), BF) for j in range(4)] for i in range(2)]
    w2_in = [[din(f"w2h{i}_{j}", (128, 16, 512), BF) for j in range(4)] for i in range(2)]
    inw_in = [din(f"inw{u}", (128, 8, 1024), BF) for u in range(4)]
    mow_in = [din(f"mow{h}", (128, 16, 512), BF) for h in range(2)]
    iw1_in = din("iw1t", (128, 8, 256), BF)
    iw2_in = din("iw2c", (128, 2), F32)
    hw1_in = din("hw1t", (128, 8, 256), BF)
    hw2_in = din("hw2c", (128, 2), F32)
    ib1_in = din("ib1c", (128, 2), F32)
    hb1_in = din("hb1c", (128, 2), F32)
    qkb_in = None if qkb_zero else din("qkbc", (128, 16), F32)
    inb_in = din("inbc", (128, 32), F32)
    gel_in = [din(f"gelc{i}", (128, 32), F32) for i in range(2)]
    dtw2_in = din("dtw2c", (128, 16), F32)
    dtb2_in = din("dtb2c", (128, 16), F32)
    dpar_in = din("dpar75c", (128, 16), F32)
    dtiw_in = din("dtinwc", (128, 16), BF)
    cwb_in = din("convwb", (128, 16, 4), BF)
    vb_in = din("vbrow", (1, D), F32) if vb_nonzero else None

    y_out = nc.dram_tensor("y_sh", [128, 4, D], F32, kind="ExternalOutput")
    dbg_t = {}
    if dbg:
        for nm, shp, dt_ in (("d_ln1", (128, 4, D), BF), ("d_impT", (1, LSH), F32),
                             ("d_ranks", (128, 4), F32),
                             ("d_tokT", (128, 8, K), BF), ("d_qkT", (128, 16, K), BF),
                             ("d_oaT", (128, 8, K), BF), ("d_xattn", (128, 4, D), F32),
                             ("d_xmlp0", (128, 4, D), F32), ("d_xmamba", (128, 4, D), F32),
                             ("d_p", (128, 4), F32)):
            dbg_t[nm] = nc.dram_tensor(nm, list(shp), dt_, kind="ExternalOutput")

    cc_imp_i = [nc.dram_tensor(f"cc_imp_i{s}", [1, LSH], F32, kind="Internal") for s in range(STEPS)]
    cc_imp_o = [nc.dram_tensor(f"cc_imp_o{s}", [4, LSH], F32, kind="Internal") for s in range(STEPS)]
    cc_tok_i = [nc.dram_tensor(f"cc_tok_i{s}", [D, K], BF, kind="Internal") for s in range(STEPS)]
    cc_tok_o = [nc.dram_tensor(f"cc_tok_o{s}", [D, K], BF, kind="Internal") for s in range(STEPS)]
    cc_tl_i = [nc.dram_tensor(f"cc_tl_i{s}", [DI, 3], BF, kind="Internal") for s in range(STEPS)]
    cc_tl_o = [nc.dram_tensor(f"cc_tl_o{s}", [4, DI, 3], BF, kind="Internal") for s in range(STEPS)]

    from contextlib import ExitStack
    with tile.TileContext(nc) as tc, ExitStack() as est:
        per = est.enter_context(tc.tile_pool(name="persist", bufs=1))
        wp = est.enter_context(tc.tile_pool(name="wpool", bufs=2))
        a1 = est.enter_context(tc.tile_pool(name="act", bufs=1))
        sp = est.enter_context(tc.tile_pool(name="small", bufs=2))
        s1 = est.enter_context(tc.tile_pool(name="small1", bufs=1))
        ps5 = est.enter_context(tc.tile_pool(name="ps5", bufs=3, space="PSUM"))
        ps2 = est.enter_context(tc.tile_pool(name="ps2", bufs=3, space="PSUM"))
        pst = est.enter_context(tc.tile_pool(name="pst", bufs=2, space="PSUM"))

        def _scopy(o, i):
            return nc.scalar.copy(o, i)

        def _vcopy(o, i):
            return nc.vector.tensor_copy(o, i)

        eng = [_scopy, _vcopy]

        # ---------- persistent / constants ----------
        x3 = per.tile([128, 4, D], F32, tag="x3")
        acc3 = per.tile([128, 4, D], F32, tag="acc3")
        rem = per.tile([128, 4], F32, tag="rem")
        idf = per.tile([128, 128], F32, tag="idf")
        idb = per.tile([128, 128], BF, tag="idb")
        iota204 = per.tile([128, K], F32, tag="iota204")
        iota_i = per.tile([128, K], I32, tag="iota_i")
        onesb = per.tile([128, 1], BF, tag="onesb")
        magic4 = per.tile([128, 4], U32, tag="magic4")
        oneu = per.tile([128, 1], U32, tag="oneu")

        for cv in sorted({0.0, 1.0, float(0.5 * b2_hp)}):
            cvt = per.tile([128, 1], F32, tag=f"cst{cv}", name=f"cst_{cv}")
            nc.vector.memset(cvt[:], cv)
            nc.const_aps.aps[(F32, cv)] = cvt[:]

        nc.sync.dma_start(x3[:], x_in[:])
        nc.vector.memset(acc3[:], 0.0)
        make_identity(nc, idf[:])
        make_identity(nc, idb[:])
        nc.gpsimd.iota(iota_i[:], pattern=[[1, K]], base=0, channel_multiplier=0)
        nc.vector.tensor_copy(iota204[:], iota_i[:])
        nc.vector.memset(onesb[:], 1.0)
        nc.vector.memset(magic4[:], MAGIC)
        nc.vector.memset(oneu[:], 1)

        selb = per.tile([128, 4], F32, tag="selb")
        iw2 = per.tile([128, 2], F32, tag="iw2")
        hw2 = per.tile([128, 2], F32, tag="hw2")
        ib1 = per.tile([128, 2], F32, tag="ib1")
        hb1 = per.tile([128, 2], F32, tag="hb1")
        inb = per.tile([128, 32], F32, tag="inb")
        gel = [per.tile([128, 32], F32, tag=f"gel{i}", name=f"gel{i}") for i in range(2)]
        dtw2 = per.tile([128, 16], F32, tag="dtw2")
        dtb2 = per.tile([128, 16], F32, tag="dtb2")
        dpar = per.tile([128, 16], F32, tag="dpar")
        dtiw = per.tile([128, 16], BF, tag="dtiw")
        cwb = per.tile([128, 16, 4], BF, tag="cwb")
        consts = [(selb, selb_in), (iw2, iw2_in), (hw2, hw2_in), (ib1, ib1_in),
                  (hb1, hb1_in), (inb, inb_in),
                  (gel[0], gel_in[0]), (gel[1], gel_in[1]),
                  (dtw2, dtw2_in), (dtb2, dtb2_in), (dpar, dpar_in), (dtiw, dtiw_in),
                  (cwb, cwb_in)]
        if not qkb_zero:
            qkb = per.tile([128, 16], F32, tag="qkb")
            consts.append((qkb, qkb_in))
        for t, d in consts:
            sdma.dma_start(t[:], d[:])
        if vb_nonzero:
            vbr = per.tile([1, D], F32, tag="vbr")
            sdma.dma_start(vbr[:], vb_in[:])

        # ---------- helpers ----------
        def ln_stats(tag, zscr, rstd1p=False):
            """LN stats of x3.  Returns st tile [128, 8, 4]:
            rows: 0=sum 1=sumsq 2=mean 3=var+eps 4=rstd 5=nmr 6=rstd1p 7=scr.
            zscr: [128, >=8192] bf16 scratch for the squares pass."""
            st = s1.tile([128, 8, 4], F32, tag=f"lnst_{tag}", name=f"lnst_{tag}")
            s_, q_ = st[:, 0, :], st[:, 1, :]
            m_, v_ = st[:, 2, :], st[:, 3, :]
            r_, nmr = st[:, 4, :], st[:, 5, :]
            scr = st[:, 7, :]
            if LN_TTR:
                # one-pass mean/var via the BN_STATS hardware op
                bns = s1.tile([128, 4, 2, 6], F32, tag=f"bns_{tag}", name=f"bns_{tag}")
                mv = s1.tile([128, 4, 2], F32, tag=f"mv_{tag}", name=f"mv_{tag}")
                for c in range(4):
                    for a in range(2):
                        nc.vector.bn_stats(bns[:, c, a, :], x3[:, c, a * 512:(a + 1) * 512])
                    nc.vector.bn_aggr(mv[:, c, :], bns[:, c, :, :])
                nc.vector.tensor_copy(m_, mv[:, :, 0])
                nc.vector.tensor_scalar_add(v_, mv[:, :, 1], EPS)
            else:
                sqf = a1.tile([128, 4, D], F32, tag="sqf", name=f"sqf_{tag}")
                nc.scalar.square(sqf[:], x3[:])
                for c in range(4):
                    nc.vector.reduce_sum(q_[:, c:c + 1], sqf[:, c, :], axis=AX.X)
                    nc.vector.reduce_sum(s_[:, c:c + 1], x3[:, c, :], axis=AX.X)
                nc.vector.tensor_scalar_mul(m_, s_, 1.0 / D)
                nc.vector.tensor_tensor(scr, m_, m_, op=OP.mult)
                nc.vector.tensor_scalar(v_, q_, 1.0 / D, EPS, op0=OP.mult, op1=OP.add)
                nc.vector.tensor_tensor(v_, v_, scr, op=OP.subtract)
            if LN_QUAKE:
                # quake rsqrt + 2 Newton iterations
                vb_ = v_.bitcast(U32)
                rb_ = r_.bitcast(U32)
                nc.vector.tensor_scalar(rb_, vb_, oneu[:, 0:1], None, op0=OP.logical_shift_right)
                nc.vector.tensor_tensor(rb_, magic4[:], rb_, op=OP.subtract)
                for _ in range(2):
                    nc.vector.tensor_tensor(scr, r_, r_, op=OP.mult)
                    nc.vector.tensor_tensor(scr, scr, v_, op=OP.mult)
                    nc.vector.tensor_scalar(scr, scr, -0.5, 1.5, op0=OP.mult, op1=OP.add)
                    nc.vector.tensor_tensor(r_, r_, scr, op=OP.mult)
            else:
                nc.scalar.activation(r_, v_, AF.Sqrt, bias=0.0, scale=1.0)
                nc.vector.reciprocal(r_, r_)
            nc.vector.scalar_tensor_tensor(nmr, m_, -1.0, r_, op0=OP.mult, op1=OP.mult)
            if rstd1p:
                nc.vector.tensor_scalar_add(st[:, 6, :], r_, 1.0)
            return st

        def ln_normalize(dst3, st):
            """dst3[:, c, :] = (x3 - mean) * rstd, bf16 out."""
            for c in range(4):
                nc.vector.tensor_scalar(dst3[:, c, :], x3[:, c, :],
                                        st[:, 4, c:c + 1], st[:, 5, c:c + 1],
                                        op0=OP.mult, op1=OP.add)

        def transpose_LD(src3, dst3, ident, ei=0):
            """src3 (128,4,D) -> dst3 (128,8,512), cast on psum->sbuf copy"""
            for dc in range(8):
                pt = pst.tile([128, 512], src3.tensor.dtype, tag="pt", name=f"pt{dc}")
                for c in range(4):
                    nc.tensor.transpose(pt[:, c * 128:(c + 1) * 128],
                                        src3[:, c, dc * 128:(dc + 1) * 128], ident[:])
                eng[(dc + ei) % 2](dst3[:, dc, :], pt[:])

        def gate_mm(tsrc, w1sb, b1col, w2col, out_cols, tanh_bias=None):
            """gate on transposed bf16 input tsrc (128,8,512):
            h=relu(w1.T@tsrc+b1) (f32); impT variant: out=[1,512] raw logits;
            halting variant: out=[128,4] sigmoid via tanh."""
            hps = [ps5.tile([128, 512], F32, tag="mm5", name=f"hps{i}") for i in range(2)]
            for dc in range(8):
                for oc in range(2):
                    nc.tensor.matmul(hps[oc][:], w1sb[:, dc, oc * 128:(oc + 1) * 128],
                                     tsrc[:, dc, :], start=dc == 0, stop=dc == 7)
            h = s1.tile([128, 2, 512], F32, tag="gateh", name="gateh")
            for oc in range(2):
                nc.scalar.activation(h[:, oc, :], hps[oc][:], AF.Relu, bias=b1col[:, oc:oc + 1])
            if out_cols.shape[-1] == 4:
                for lc in range(4):
                    ps = ps2.tile([128, K], F32, tag="mm2", name=f"gp{lc}")
                    for oc in range(2):
                        nc.tensor.matmul(ps[:, 0:1], h[:, oc, lc * 128:(lc + 1) * 128],
                                         w2col[:, oc:oc + 1], start=oc == 0, stop=oc == 1)
                    # sigmoid(z+b2) = 0.5 + 0.5*tanh((z+b2)/2)
                    nc.scalar.activation(out_cols[:, lc:lc + 1], ps[:, 0:1], AF.Tanh,
                                         bias=float(tanh_bias), scale=0.5)
                nc.vector.tensor_scalar(out_cols[:], out_cols[:], 0.5, 0.5,
                                        op0=OP.mult, op1=OP.add)
            else:
                ps = ps5.tile([128, 512], F32, tag="mm5", name="gpi")
                for oc in range(2):
                    nc.tensor.matmul(ps[:1, :], w2col[:, oc:oc + 1], h[:, oc, :],
                                     start=oc == 0, stop=oc == 1)
                nc.vector.tensor_copy(out_cols[:], ps[:1, :])

        # ================= per-step program =================
        for s in range(STEPS):
            zT = a1.tile([128, 16, 512], BF, tag="m16b", name=f"zT_{s}")
            zflat = _r(zT[:], "p a b -> p (a b)")

            # ---------- block 0 : selective attention ----------
            iw1 = wp.tile([128, 8, 256], BF, tag="wsmall", name=f"iw1_{s}")
            nc.sync.dma_start(iw1[:], iw1_in[:])

            st1 = ln_stats(f"ln1_{s}", zflat, rstd1p=True)
            tTn = a1.tile([128, 8, 512], BF, tag="tTn", name=f"tTn_{s}")
            junkb = a1.tile([128, 4, D], BF, tag="junkb", name=f"junkb_{s}")
            ln_normalize(junkb, st1)
            transpose_LD(junkb, tTn, idb[:])
            if dbg and s == 0:
                gdma.dma_start(dbg_t["d_ln1"][:], junkb[:])

            impT = sp.tile([1, 512], F32, tag="impT", name=f"impT_{s}")
            gate_mm(tTn, iw1, ib1, iw2, impT)

            gdma.dma_start(cc_imp_i[s][:], impT[:])
            if nocc:
                for g_ in range(4):
                    gdma.dma_start(cc_imp_o[s][g_:g_ + 1, :], cc_imp_i[s][:])
            else:
                nc.gpsimd.collective_compute("AllGather", OP.bypass, replica_groups=GROUPS,
                                             ins=[cc_imp_i[s][:]], outs=[cc_imp_o[s][:]])

            # prefetch attention weights during the collective
            wqk = [wp.tile([128, 8, 1024], BF, tag="wbig", name=f"wqk{h}_{s}") for h in range(2)]
            nc.sync.dma_start(wqk[0][:], wqk_in[0][:])
            nc.sync.dma_start(wqk[1][:], wqk_in[1][:])

            if RES_NEW:
                # fold ln1(x) residual into x3 now: x3 = x3*(1+rstd) + nmr
                # (equals x + ln1(x); scat psum is added later)
                for c in range(4):
                    nc.vector.tensor_scalar(x3[:, c, :], x3[:, c, :],
                                            st1[:, 6, c:c + 1], st1[:, 5, c:c + 1],
                                            op0=OP.mult, op1=OP.add)

            imp_bc = a1.tile([128, L], F32, tag="uT", name=f"impbc_{s}")
            sdma.dma_start(imp_bc[:],
                                _r(cc_imp_o[s][:], "g l -> (g l)").unsqueeze(0).broadcast_to([128, L]))

            if dbg and s == 0:
                gdma.dma_start(dbg_t["d_impT"][:], impT[:])
            pic = pst.tile([128, 512], F32, tag="pt", name=f"pic_{s}")
            for c in range(4):
                nc.tensor.transpose(pic[:, c:c + 1], impT[:1, c * 128:(c + 1) * 128],
                                    idf[:1, :1])
            icols = sp.tile([128, 4], F32, tag="icols", name=f"icols_{s}")
            nc.vector.tensor_copy(icols[:], pic[:, 0:4])
            ranks = sp.tile([128, 4], F32, tag="ranks", name=f"ranks_{s}")
            for c in range(4):
                nc.vector.tensor_scalar(zflat[:, c * L:(c + 1) * L], imp_bc[:],
                                        icols[:, c:c + 1], None, op0=OP.is_gt,
                                        op1=OP.add, accum_out=ranks[:, c:c + 1])

            if dbg and s == 0:
                gdma.dma_start(dbg_t["d_ranks"][:], ranks[:])
            # selection matrices
            ST = a1.tile([128, 4, K], BF, tag="ST", name=f"ST_{s}")
            for c in range(4):
                nc.vector.tensor_scalar(ST[:, c, :], iota204[:], ranks[:, c:c + 1], None,
                                        op0=OP.is_equal)
            S_ = a1.tile([128, 2, 512], BF, tag="S_", name=f"S__{s}")
            for c in range(4):
                pt = pst.tile([128, 2, 128], BF, tag="pt", name=f"ptS{c}")
                nc.tensor.transpose(pt[:, 0, :], ST[:, c, 0:128], idb[:])
                nc.tensor.transpose(pt[:76, 1, :], ST[:, c, 128:K], idb[:])
                nc.vector.tensor_copy(S_[:, :, c * 128:(c + 1) * 128], pt[:])

            # gather own tokens -> bf16 partial tokT, AllReduce
            tokp = a1.tile([128, 8, K], BF, tag="gT", name=f"tokp_{s}")
            for dc in range(8):
                ps = ps2.tile([128, K], F32, tag="mm2", name=f"tg{dc}")
                for c in range(4):
                    nc.tensor.matmul(ps[:], junkb[:, c, dc * 128:(dc + 1) * 128],
                                     ST[:, c, :], start=c == 0, stop=c == 3)
                eng[dc % 2](tokp[:, dc, :], ps[:])
            gdma.dma_start(_r(cc_tok_i[s][:], "(dc p) k -> p dc k", p=128), tokp[:])
            if nocc:
                gdma.dma_start(cc_tok_o[s][:], cc_tok_i[s][:])
            else:
                nc.gpsimd.collective_compute("AllReduce", OP.add, replica_groups=GROUPS,
                                             ins=[cc_tok_i[s][:]], outs=[cc_tok_o[s][:]])
            # prefetch wv during the AllReduce
            wv = wp.tile([128, 8, D], BF, tag="wbig", name=f"wv_{s}")
            nc.sync.dma_start(wv[:], wv_in[:])
            tokT = a1.tile([128, 8, K], BF, tag="m16a", name=f"tokT_{s}")
            sdma.dma_start(tokT[:], _r(cc_tok_o[s][:], "(dc p) k -> p dc k", p=128))

            if dbg and s == 0:
                gdma.dma_start(dbg_t["d_tokT"][:], tokT[:])
            # qkT (+bias)
            qkT = a1.tile([128, 16, K], BF, tag="qkT", name=f"qkT_{s}")
            for half in range(2):
                for oc in range(8):
                    ps = ps2.tile([128, K], F32, tag="mm2", name=f"qk{oc}")
                    for kc in range(8):
                        nc.tensor.matmul(ps[:], wqk[half][:, kc, oc * 128:(oc + 1) * 128],
                                         tokT[:, kc, :], start=kc == 0, stop=kc == 7)
                    if qkb_zero:
                        eng[oc % 2](qkT[:, half * 8 + oc, :], ps[:])
                    else:
                        nc.scalar.activation(qkT[:, half * 8 + oc, :], ps[:], AF.Identity,
                                             bias=qkb[:, half * 8 + oc:half * 8 + oc + 1])
            if dbg and s == 0:
                gdma.dma_start(dbg_t["d_qkT"][:], qkT[:])
            # v
            v_ = [a1.tile([rn, D], BF, tag=f"v{i}", name=f"v{i}_{s}") for i, (rs, rn) in enumerate(KC2)]
            for i, (rs, rn) in enumerate(KC2):
                for hf in range(2):
                    ps = ps5.tile([128, 512], F32, tag="mm5", name=f"vp{i}{hf}")
                    for dc in range(8):
                        nc.tensor.matmul(ps[:rn, :], tokT[:, dc, rs:rs + rn],
                                         wv[:, dc, hf * 512:(hf + 1) * 512],
                                         start=dc == 0, stop=dc == 7)
                    if vb_nonzero:
                        nc.vector.tensor_add(ps[:rn, :], ps[:rn, :],
                                             vbr[:1, hf * 512:(hf + 1) * 512].broadcast_to([rn, 512]))
                    eng[(i + hf) % 2](v_[i][:, hf * 512:(hf + 1) * 512], ps[:rn, :])
            # prefetch wo during the head loop
            wo = wp.tile([128, 8, D], BF, tag="wbig", name=f"wo_{s}")
            nc.sync.dma_start(wo[:], wo_in[:])

            # attention heads: transposed scores, exp w/o max-sub,
            # normalize after the V contraction
            oaT = a1.tile([128, 8, K], BF, tag="oaT", name=f"oaT_{s}")
            for h in range(NH if ATT_NEW else 0):
                hp_, hs = h // 2, 64 * (h % 2)
                eT = [sp.tile([rn, K], BF, tag=f"eT{i}", name=f"eT{i}_{s}_{h}")
                      for i, (rs, rn) in enumerate(KC2)]
                # allocation order matters: the ps2 ring has 3 buffers, so the
                # 4th tile (po) wraps onto sc[0]'s buffer — sc[0] is released
                # by exp0 before po's first matmul needs eT0, so no cycle.
                # (cs/po allocated first would wrap a LIVE accumulator: deadlock.)
                scs = [ps2.tile([128, K], F32, tag="mm2", name=f"sc{i}_{h}")
                       for i in range(2)]
                cs = ps2.tile([128, K], F32, tag="mm2", name=f"cs_{h}")
                po = ps2.tile([128, K], F32, tag="mm2", name=f"po_{h}")
                for i, (rs, rn) in enumerate(KC2):
                    nc.tensor.matmul(scs[i][:rn, :], qkT[hs:hs + 64, 8 + hp_, rs:rs + rn],
                                     qkT[hs:hs + 64, hp_, :], start=True, stop=True)
                    nc.scalar.activation(eT[i][:], scs[i][:rn, :], AF.Exp)
                    nc.tensor.matmul(cs[:1, :], onesb[:rn, 0:1], eT[i][:],
                                     start=i == 0, stop=i == 1)
                    nc.tensor.matmul(po[:64, :], v_[i][:, h * 64:(h + 1) * 64],
                                     eT[i][:], start=i == 0, stop=i == 1)
                csr = sp.tile([64, K], F32, tag="csr", name=f"csr_{s}_{h}")
                nc.vector.reciprocal(csr[:1, :], cs[:1, :])
                nc.gpsimd.partition_broadcast(csr[:], csr[:1, :])
                nc.vector.tensor_tensor(oaT[hs:hs + 64, hp_, :], po[:64, :],
                                        csr[:], op=OP.mult)
            for h in range(0 if ATT_NEW else NH):
                # fallback: baseline softmax (max-sub, attb/attT transposes)
                hp_, hs = h // 2, 64 * (h % 2)
                attb = [sp.tile([rn, K], BF, tag=f"eT{i}", name=f"attb{i}_{s}_{h}")
                        for i, (rs, rn) in enumerate(KC2)]
                for i, (qs, qn) in enumerate(KC2):
                    ps = ps2.tile([128, K], F32, tag="mm2", name=f"sc{i}_{h}")
                    nc.tensor.matmul(ps[:qn, :], qkT[hs:hs + 64, hp_, qs:qs + qn],
                                     qkT[hs:hs + 64, 8 + hp_, :], start=True, stop=True)
                    mx = sp.tile([128, 2], F32, tag="mx", name=f"mx{i}_{s}_{h}")
                    nc.vector.tensor_reduce(mx[:qn, 0:1], ps[:qn, :], axis=AX.X,
                                            op=OP.max, negate=True)
                    nc.scalar.activation(attb[i][:], ps[:qn, :], AF.Exp,
                                         bias=mx[:qn, 0:1], scale=1.0,
                                         accum_out=mx[:qn, 1:2])
                    nc.vector.reciprocal(mx[:qn, 0:1], mx[:qn, 1:2])
                    nc.vector.tensor_scalar_mul(attb[i][:], attb[i][:], mx[:qn, 0:1])
                attT = sp.tile([128, 2, K], BF, tag="attT", name=f"attT_{s}_{h}")
                pt = pst.tile([128, 2, K], BF, tag="pt", name=f"ptA_{s}_{h}")
                nc.tensor.transpose(pt[:, 0, 0:128], attb[0][:, 0:128], idb[:])
                nc.tensor.transpose(pt[:76, 1, 0:128], attb[0][:, 128:K], idb[:])
                nc.tensor.transpose(pt[:, 0, 128:K], attb[1][:, 0:128], idb[:76, :76])
                nc.tensor.transpose(pt[:76, 1, 128:K], attb[1][:, 128:K], idb[:76, :76])
                nc.vector.tensor_copy(attT[:], pt[:])
                po = ps2.tile([128, K], F32, tag="mm2", name=f"po_{h}")
                nc.tensor.matmul(po[:64, :], v_[0][:, h * 64:(h + 1) * 64],
                                 attT[:, 0, :], start=True, stop=False)
                nc.tensor.matmul(po[:64, :], v_[1][:76, h * 64:(h + 1) * 64],
                                 attT[:76, 1, :], start=False, stop=True)
                eng[h % 2](oaT[hs:hs + 64, hp_, :], po[:64, :])

            if dbg and s == 0:
                gdma.dma_start(dbg_t["d_oaT"][:], oaT[:])
            # output projection + scatter + residual
            o_ = [a1.tile([rn, D], BF, tag=f"o{i}", name=f"o{i}_{s}") for i, (rs, rn) in enumerate(KC2)]
            for i, (rs, rn) in enumerate(KC2):
                for hf in range(2):
                    ps = ps5.tile([128, 512], F32, tag="mm5", name=f"op{i}{hf}")
                    for dc in range(8):
                        nc.tensor.matmul(ps[:rn, :], oaT[:, dc, rs:rs + rn],
                                         wo[:, dc, hf * 512:(hf + 1) * 512],
                                         start=dc == 0, stop=dc == 7)
                    eng[(i + hf) % 2](o_[i][:, hf * 512:(hf + 1) * 512], ps[:rn, :])
            for lc in range(4):
                for hf in range(2):
                    ps = ps5.tile([128, 512], F32, tag="mm5", name=f"scat{lc}{hf}")
                    nc.tensor.matmul(ps[:], S_[:, 0, lc * 128:(lc + 1) * 128],
                                     o_[0][:, hf * 512:(hf + 1) * 512], start=True, stop=False)
                    nc.tensor.matmul(ps[:], S_[:76, 1, lc * 128:(lc + 1) * 128],
                                     o_[1][:76, hf * 512:(hf + 1) * 512], start=False, stop=True)
                    if RES_NEW:
                        # x3 already holds x + ln1(x); just add the scatter
                        nc.vector.tensor_add(x3[:, lc, hf * 512:(hf + 1) * 512],
                                             x3[:, lc, hf * 512:(hf + 1) * 512], ps[:])
                    else:
                        nc.vector.tensor_add(ps[:], ps[:],
                                             junkb[:, lc, hf * 512:(hf + 1) * 512])
                        nc.vector.tensor_add(x3[:, lc, hf * 512:(hf + 1) * 512],
                                             x3[:, lc, hf * 512:(hf + 1) * 512], ps[:])

            if dbg and s == 0:
                gdma.dma_start(dbg_t["d_xattn"][:], x3[:])
            # ---------- MLP (after both mixers) ----------
            def mlp(bi):
                st2 = ln_stats(f"ln2_{s}_{bi}", zflat)
                uT = a1.tile([128, 8, 512], BF, tag="uT", name=f"uT_{s}_{bi}")
                junk2 = a1.tile([128, 4, D], BF, tag="junkb", name=f"junk2_{s}_{bi}")
                ln_normalize(junk2, st2)
                transpose_LD(junk2, uT, idb[:])
                for half in range(2):
                    gT = a1.tile([128, 16, 512], BF, tag="gT", name=f"gT_{s}_{bi}_{half}")
                    for q in range(2):
                        w1u = wp.tile([128, 8, 1024], BF, tag="wbig",
                                      name=f"w1u_{s}_{bi}_{half}_{q}")
                        nc.sync.dma_start(w1u[:], w1_in[bi][half * 2 + q][:])
                        for oc in range(8):
                            ps = ps5.tile([128, 512], F32, tag="mm5", name=f"up{oc}")
                            for kc in range(8):
                                nc.tensor.matmul(ps[:], w1u[:, kc, oc * 128:(oc + 1) * 128],
                                                 uT[:, kc, :], start=kc == 0, stop=kc == 7)
                            och = half * 16 + q * 8 + oc
                            nc.scalar.activation(gT[:, q * 8 + oc, :], ps[:], AF.Gelu,
                                                 bias=gel[bi][:, och:och + 1])
                    for hf in range(2):
                        w2u = wp.tile([128, 16, 512], BF, tag="wbig",
                                      name=f"w2u_{s}_{bi}_{half}_{hf}")
                        nc.sync.dma_start(w2u[:], w2_in[bi][half * 2 + hf][:])
                        for lc in range(4):
                            ps = ps5.tile([128, 512], F32, tag="mm5", name=f"dn{lc}")
                            for kc in range(16):
                                nc.tensor.matmul(ps[:], gT[:, kc, lc * 128:(lc + 1) * 128],
                                                 w2u[:, kc, :], start=kc == 0, stop=kc == 15)
                            nc.vector.tensor_add(x3[:, lc, hf * 512:(hf + 1) * 512],
                                                 x3[:, lc, hf * 512:(hf + 1) * 512], ps[:])

            mlp(0)

            if dbg and s == 0:
                gdma.dma_start(dbg_t["d_xmlp0"][:], x3[:])
            # conv diagonal taps for this step; lives in the gT slot, which is
            # dead between mlp(0) and mlp(1).  Generated on GpSimd during the
            # mamba ln/in-projection window.
            dgall = a1.tile([128, 64, 128], BF, tag="gT", name=f"dgall_{s}")
            for kc in range(16):
                for j in range(4):
                    nc.gpsimd.affine_select(
                        dgall[:, kc * 4 + j, :], cwb[:, kc, j:j + 1].broadcast_to([128, 128]),
                        pattern=[[-1, 128]], compare_op=OP.is_equal, fill=0.0,
                        base=0, channel_multiplier=1)

            # ---------- block 1 : mamba ----------
            st3 = ln_stats(f"ln1b_{s}", zflat)
            t2T = a1.tile([128, 8, 512], BF, tag="uT", name=f"t2T_{s}")
            junk3 = a1.tile([128, 4, D], BF, tag="junkb", name=f"junk3_{s}")
            ln_normalize(junk3, st3)
            transpose_LD(junk3, t2T, idb[:])

            xiE = a1.tile([128, 16, 515], BF, tag="m16a", name=f"xiE_{s}")
            tailsb = sp.tile([128, 16, 3], BF, tag="tailsb", name=f"tailsb_{s}")
            inwu = {}
            # halo tails first so the AllGather overlaps the whole in-projection
            for u in range(2):
                inwu[u] = wp.tile([128, 8, 1024], BF, tag="wbig", name=f"inw{u}_{s}")
                nc.sync.dma_start(inwu[u][:], inw_in[u][:])
                for oc8 in range(8):
                    oc = u * 8 + oc8
                    pt3 = ps2.tile([128, K], F32, tag="mm2", name=f"tl{u}_{oc8}")
                    for kc in range(8):
                        nc.tensor.matmul(pt3[:, 0:3],
                                         inwu[u][:, kc, oc8 * 128:(oc8 + 1) * 128],
                                         t2T[:, kc, 509:512],
                                         start=kc == 0, stop=kc == 7)
                    if inb_zero:
                        nc.vector.tensor_copy(tailsb[:, oc, :], pt3[:, 0:3])
                    else:
                        nc.scalar.activation(tailsb[:, oc, :], pt3[:, 0:3], AF.Identity,
                                             bias=inb[:, oc:oc + 1])
            gdma.dma_start(_r(cc_tl_i[s][:], "(k p) j -> p k j", p=128), tailsb[:])
            if nocc:
                for g_ in range(4):
                    gdma.dma_start(cc_tl_o[s][g_], cc_tl_i[s][:])
            else:
                nc.gpsimd.collective_compute(
                    "AllGather", OP.bypass, replica_groups=GROUPS,
                    ins=[cc_tl_i[s][:]], outs=[cc_tl_o[s][:]])

            for u in range(4):
                if u >= 2:
                    inwu[u] = wp.tile([128, 8, 1024], BF, tag="wbig", name=f"inw{u}_{s}")
                    nc.sync.dma_start(inwu[u][:], inw_in[u][:])
                for oc8 in range(8):
                    oc = u * 8 + oc8
                    ps = ps5.tile([128, 512], F32, tag="mm5", name=f"ip{oc8}")
                    for kc in range(8):
                        nc.tensor.matmul(ps[:], inwu[u][:, kc, oc8 * 128:(oc8 + 1) * 128],
                                         t2T[:, kc, :], start=kc == 0, stop=kc == 7)
                    if oc < 16:
                        if inb_zero:
                            eng[oc % 2](xiE[:, oc, 3:515], ps[:])
                        else:
                            nc.scalar.activation(xiE[:, oc, 3:515], ps[:], AF.Identity,
                                                 bias=inb[:, oc:oc + 1])
                    else:
                        nc.scalar.activation(zT[:, oc - 16, :], ps[:], AF.Silu,
                                             bias=inb[:, oc:oc + 1])

            tails = sp.tile([128, 16, 4, 3], BF, tag="tails", name=f"tails_{s}")
            for g_ in range(4):
                sdma.dma_start(tails[:, :, g_, :],
                                    _r(cc_tl_o[s][:], "g (k p) j -> g p k j", p=128)[g_])
            htmp = sp.tile([128, 16, 3, 4], F32, tag="htmp", name=f"htmp_{s}")
            hsum = sp.tile([128, 16, 3], F32, tag="hsum", name=f"hsum_{s}")
            nc.vector.tensor_mul(htmp[:], tails[:].transpose([0, 1, 3, 2]),
                                 selb[:].unsqueeze(1).unsqueeze(1).broadcast_to([128, 16, 3, 4]))
            nc.vector.reduce_sum(hsum[:], htmp[:], axis=AX.X)
            nc.gpsimd.tensor_copy(xiE[:, :, 0:3], hsum[:])

            # depthwise causal conv as 4 diagonal-matmul taps + silu
            for kc in range(16):
                pc = ps5.tile([128, 512], F32, tag="mm5", name=f"cv_{kc}")
                for j in range(4):
                    dg = dgall[:, kc * 4 + j, :]
                    nc.tensor.matmul(pc[:, 3:512], dg, xiE[:, kc, 3 + j:512 + j],
                                     start=j == 0, stop=False)
                    nc.tensor.matmul(pc[:, 0:3], dg, xiE[:, kc, j:3 + j],
                                     start=j == 0, stop=j == 3)
                nc.scalar.activation(xiE[:, kc, 3:515], pc[:], AF.Silu)

            # dt path + gating (result written into zT)
            # sigmoid(softplus(z)) + D = (0.75+D) - 0.25*tanh((ln2 - z)/2)
            psd = ps5.tile([128, 512], F32, tag="mm5", name=f"dtin_{s}")
            for kc in range(16):
                nc.tensor.matmul(psd[:1, :], dtiw[:, kc:kc + 1], xiE[:, kc, 3:515],
                                 start=kc == 0, stop=kc == 15)
            dt_bc = s1.tile([128, 512], F32, tag="dtbc", name=f"dtbc_{s}")
            nc.vector.tensor_copy(dt_bc[:1, :], psd[:1, :])
            nc.gpsimd.partition_broadcast(dt_bc[:], dt_bc[:1, :])
            for kc in range(16):
                dsp = sp.tile([128, 512], F32, tag="dsp", name=f"dsp_{s}_{kc}")
                nc.scalar.activation(dsp[:], dt_bc[:], AF.Tanh,
                                     bias=dtb2[:, kc:kc + 1], scale=dtw2[:, kc:kc + 1])
                nc.vector.tensor_scalar(dsp[:], dsp[:], -0.25, dpar[:, kc:kc + 1],
                                        op0=OP.mult, op1=OP.add)
                nc.vector.tensor_mul(dsp[:], dsp[:], xiE[:, kc, 3:515])
                nc.gpsimd.tensor_mul(zT[:, kc, :], dsp[:], zT[:, kc, :])

            for hf in range(2):
                mowu = wp.tile([128, 16, 512], BF, tag="wbig", name=f"mow{hf}_{s}")
                nc.sync.dma_start(mowu[:], mow_in[hf][:])
                for lc in range(4):
                    ps = ps5.tile([128, 512], F32, tag="mm5", name=f"mo{lc}")
                    for kc in range(16):
                        nc.tensor.matmul(ps[:], zT[:, kc, lc * 128:(lc + 1) * 128],
                                         mowu[:, kc, :], start=kc == 0, stop=kc == 15)
                    nc.vector.tensor_add(x3[:, lc, hf * 512:(hf + 1) * 512],
                                         x3[:, lc, hf * 512:(hf + 1) * 512], ps[:])

            if dbg and s == 0:
                gdma.dma_start(dbg_t["d_xmamba"][:], x3[:])
            mlp(1)

            # ---------- halting gate (last step's gate is algebraically dead) ----------
            if s < STEPS - 1:
                hw1 = wp.tile([128, 8, 256], BF, tag="wsmall", name=f"hw1_{s}")
                nc.sync.dma_start(hw1[:], hw1_in[:])
                tTh = a1.tile([128, 8, 512], BF, tag="tTn", name=f"tTh_{s}")
                transpose_LD(x3, tTh, idf[:])
                p_ = sp.tile([128, 4], F32, tag="pcol", name=f"p_{s}")
                gate_mm(tTh, hw1, hb1, hw2, p_, tanh_bias=0.5 * b2_hp)
                if dbg and s == 0:
                    gdma.dma_start(dbg_t["d_p"][:], p_[:])
                # halt starts at 0 < THRESH, rem starts at 1, p = sigmoid < 1:
                # nh = p; rem = 1 - p; acc += p * x
                nc.vector.tensor_scalar(rem[:], p_[:], -1.0, 1.0, op0=OP.mult, op1=OP.add)
                for c in range(4):
                    nc.vector.scalar_tensor_tensor(acc3[:, c, :], x3[:, c, :],
                                                   p_[:, c:c + 1], acc3[:, c, :],
                                                   op0=OP.mult, op1=OP.add)

        # ---------- final output: y = acc + rem * x ----------
        for c in range(4):
            nc.vector.scalar_tensor_tensor(acc3[:, c, :], x3[:, c, :], rem[:, c:c + 1],
                                           acc3[:, c, :], op0=OP.mult, op1=OP.add)
        nc.sync.dma_start(y_out[:], acc3[:])

    nc.compile()
    return nc


_CACHE = {}


def _get_program(b2_hp, vb_nonzero, qkb_zero=False, inb_zero=False):
    key = (round(float(b2_hp), 9), bool(vb_nonzero), qkb_zero, inb_zero)
    if key not in _CACHE:
        _CACHE[key] = build_program(float(b2_hp), bool(vb_nonzero),
                                    qkb_zero=qkb_zero, inb_zero=inb_zero)
    return _CACHE[key]


def _cols(a, n=128):
    """(n*k,) -> (n, k) column-chunk layout"""
    a = np.asarray(a, np.float32).reshape(-1, n)
    return np.ascontiguousarray(a.T)


def _t8(w):
    """(1024, O) -> [128, 8, O] matching '(k p) o -> p k o'"""
    return np.ascontiguousarray(w.reshape(8, 128, -1).transpose(1, 0, 2))


def _t16(w):
    """(2048, O) -> [128, 16, O]"""
    return np.ascontiguousarray(w.reshape(16, 128, -1).transpose(1, 0, 2))


def prepare_inputs(inp):
    g = lambda k: np.asarray(inp[k], np.float32)
    x = g("x")

    def fold(w, gam, bet):
        return w * gam[None, :], w @ bet

    iw1, ib1 = fold(g("b0_imp_w1"), g("b0_ln1_g"), g("b0_ln1_b"))
    ib1 = ib1 + g("b0_imp_b1")
    qkv, qkvb = fold(g("b0_qkv_w"), g("b0_ln1_g"), g("b0_ln1_b"))
    qkv = qkv.copy()
    qkvb = qkvb.copy()
    qkv[:D] /= np.sqrt(HD)
    qkvb[:D] /= np.sqrt(HD)
    w10, gel0 = fold(g("b0_mlp_w1"), g("b0_ln2_g"), g("b0_ln2_b"))
    inw, inb = fold(g("b1_in_w"), g("b1_ln1_g"), g("b1_ln1_b"))
    w11, gel1 = fold(g("b1_mlp_w1"), g("b1_ln2_g"), g("b1_ln2_b"))
    wo = float(g("b0_res_w")[0]) * g("b0_out_w")

    vb = qkvb[2 * D:]
    vb_nonzero = bool(np.any(vb != 0.0))

    qkvT = np.ascontiguousarray(qkv[:2 * D].T)       # (D, 2048)
    w1T0 = np.ascontiguousarray(w10.T)               # (D, 4096)
    w1T1 = np.ascontiguousarray(w11.T)
    w2T0 = np.ascontiguousarray(g("b0_mlp_w2").T)    # (4096, D)
    w2T1 = np.ascontiguousarray(g("b1_mlp_w2").T)
    inwT = np.ascontiguousarray(inw.T)               # (D, 4096)
    mowT = np.ascontiguousarray(g("b1_out_w").T)     # (2048, D)

    wdict = {
        "wqk0": _t8(qkvT[:, :1024]).astype(bf16),
        "wqk1": _t8(qkvT[:, 1024:]).astype(bf16),
        "wv_t": _t8(np.ascontiguousarray(qkv[2 * D:].T)).astype(bf16),
        "wo_t": _t8(np.ascontiguousarray(wo.T)).astype(bf16),
        "iw1t": _t8(np.ascontiguousarray(iw1.T)).astype(bf16),
        "hw1t": _t8(np.ascontiguousarray(g("hp_w1").T)).astype(bf16),
        "iw2c": _cols(g("b0_imp_w2")[0]),
        "hw2c": _cols(g("hp_w2")[0]),
        "ib1c": _cols(ib1),
        "hb1c": _cols(g("hp_b1")),
        "inbc": _cols(inb),
        "gelc0": _cols(gel0),
        "gelc1": _cols(gel1),
        "dtw2c": _cols(-0.5 * g("b1_dt_w")[:, 0]),
        "dtb2c": _cols(0.5 * (LN2C - g("b1_dt_b"))),
        "dpar75c": _cols(0.75 + g("b1_D")),
        "dtinwc": _cols(g("b1_xproj_w")[2 * 16]).astype(bf16),
        "convwb": np.ascontiguousarray(
            g("b1_conv_w").reshape(DI, DC).reshape(16, 128, DC).transpose(1, 0, 2)).astype(bf16),
    }
    for j in range(4):
        wdict[f"w1q0_{j}"] = _t8(w1T0[:, j * 1024:(j + 1) * 1024]).astype(bf16)
        wdict[f"w1q1_{j}"] = _t8(w1T1[:, j * 1024:(j + 1) * 1024]).astype(bf16)
        wdict[f"inw{j}"] = _t8(inwT[:, j * 1024:(j + 1) * 1024]).astype(bf16)
    for half in range(2):
        for hf in range(2):
            j = half * 2 + hf
            wdict[f"w2h0_{j}"] = _t16(
                w2T0[half * 2048:(half + 1) * 2048, hf * 512:(hf + 1) * 512]).astype(bf16)
            wdict[f"w2h1_{j}"] = _t16(
                w2T1[half * 2048:(half + 1) * 2048, hf * 512:(hf + 1) * 512]).astype(bf16)
        wdict[f"mow{half}"] = _t16(
            mowT[:, half * 512:(half + 1) * 512]).astype(bf16)

    qkb_zero = bool(not np.any(qkvb[:2 * D]))
    if not qkb_zero:
        wdict["qkbc"] = _cols(qkvb[:2 * D])
    if vb_nonzero:
        wdict["vbrow"] = np.ascontiguousarray(vb[None, :])
    inb_zero = bool(not np.any(inb))

    in_maps = []
    for core in range(NC_):
        b, gp = core // 4, core % 4
        sel = np.zeros(4, np.float32)
        if gp > 0:
            sel[gp - 1] = 1.0
        m = dict(wdict)
        m["x_sh"] = np.ascontiguousarray(
            x[b, gp * LSH:(gp + 1) * LSH].reshape(4, 128, D).transpose(1, 0, 2))
        m["selb"] = np.ascontiguousarray(np.broadcast_to(sel, (128, 4)).copy())
        in_maps.append(m)
    return in_maps, float(g("hp_b2")[0]), vb_nonzero, (qkb_zero, inb_zero)


def kernel(**inputs):
    in_maps, b2_hp, vb_nonzero, flags = prepare_inputs(inputs)
    nc = _get_program(b2_hp, vb_nonzero, *flags)
    res = bass_utils.run_bass_kernel_spmd(nc, in_maps, core_ids=list(range(NC_)))
    y = np.zeros((B, L, D), np.float32)
    for core in range(NC_):
        b, gp = core // 4, core % 4
        y[b, gp * LSH:(gp + 1) * LSH] = res.results[core]["y_sh"].transpose(1, 0, 2).reshape(LSH, D)
    return y
